# revision 13
# baseline (speedup 1.0000x reference)
"""BFP (block floating point) activation quantization kernel for Trainium2.

Problem: x [32, 256, 56, 56] f32; per (batch, 32-channel block, h, w) the 32
channels share an exponent e = floor(log2(max |x|)); quantize each value to
q * 2^(e-2) with q = clip(round(x / 2^(e-2)), -7, 7)  (mantissa=3 bits).

Strategy (pure data-parallel over batch, 4 images per core on 8 cores):
  - The host permutes each core's shard into the SBUF layout
    [chunk 8][p 128][j 98][ci 32] (p = img*32 + blk*4 + hwq), so every DMA
    is a fully linear transfer AND the 32 channels of each block are the
    contiguous innermost run: the whole maxabs tree collapses into ONE
    vector.tensor_reduce(axis=X) per chunk (bf16 2x, input-bound).
  - Per chunk the engines split:
      ScalarE: w = |x| -> bf16 (the only full pass; exponent survives)
      Vector:  m32 = reduce_max(w) over ci (1 op), then ONE 8-stage custom
               DVE op computes the quantized output in a single f32 pass
               straight from m32:
                   t   = bits(m32) & POS_INF  (= 2^e; hard-wired +Inf mask)
                   Mb  = t * 3145728          (= MAGIC * 2^(e-2), exact)
                   B   = t * 1.8              (= 7.2*scale; any bound in
                                               (6.5, 7.5)*scale is exact)
                   out = (clip(x, -B, B) + Mb) - Mb    -> fp8 e4m3
               (the f32 +Mb add RNE-rounds to the scale grid whose step is
               the ulp at Mb's binade; -Mb is exact by Sterbenz)
      GpSimd:  only SWDGE store descriptor generation.
  - fp8 e4m3 output is exact (q has <= 3 mantissa bits, e in [-7,7] here)
    and halves store traffic; the host un-permutes/upconverts.
  - Chunk 0 and chunk 7 are processed in flat j-halves (the (j,ci) layout
    makes halves contiguous) to cut pipeline ramp-in and drain; the last
    chunk stores via the SP HWDGE ring right after each half's custom op.
"""

import numpy as np

import concourse.bass as bass
import concourse.tile as tile
from concourse import bacc, mybir
from concourse import dve_ops as _DO
from concourse.bass_utils import run_bass_kernel_spmd
from concourse.dve_spec import (
    C0, C1, Bin, Leaf, Spec, Src0, Src1, Zero, lower, maxx, minn,
)
from concourse.dve_uop import AluOp, DveOpSpec, InpSel

F32 = mybir.dt.float32
BF16 = mybir.dt.bfloat16
FP8 = mybir.dt.float8e4
I32 = mybir.dt.int32

N_CORES = 8
B, C, H, W = 32, 256, 56, 56
HW = H * W            # 3136
BPC = B // N_CORES    # 4 images per core
NCHUNK = 8
J = HW // 4 // NCHUNK  # 98 hw-inner elements per chunk
CI = 32               # channels per block
FREE = CI * J         # free elems per chunk per partition
MAGIC = 12582912.0    # 1.5 * 2**23: RNE round-to-integer magic for |v| < 2**22

_CACHE = {}


def _register_bfp_op():
    """Custom DVE op: the entire BFP quantization in one 8-stage pass.

        t   = in1 & 0x7F800000   (AND with the hard-wired +Inf source;
                                  in1 = maxabs > 0, so t = 2^e exactly)
        Mb  = t * s0             (s0 = 3145728 = MAGIC/4 -> MAGIC*2^(e-2);
                                  exact: power-of-two times 1.5*2^21)
        B   = t * s1             (s1 = 1.8 -> 7.2*2^(e-2); any bound in
                                  (6.5, 7.5)*scale gives exact clipping)
        out = (clip(in0, -B, B) + Mb) - Mb
    """
    name = "BFP_FUSED_QUANT"
    for op in _DO.OPS:
        if op.name == name:
            return op

    def _ref(in0, in1, s0, s1, imm2):
        # per-stage f32 rounding is load-bearing (the magic-add trick)
        f32 = np.float32
        tb = (in1.astype(f32).view(np.uint32) & 0x7F800000).view(f32)
        mb = (tb * f32(s0)).astype(f32)
        bb = (tb * f32(s1)).astype(f32)
        v = np.minimum(np.maximum(in0.astype(f32), -bb), bb).astype(f32)
        r = (v + mb).astype(f32)
        return (r - mb).astype(f32)

    texp = Bin(AluOp.BITWISE_AND, Src1, Leaf(InpSel.POS_INF))
    mb_n = texp * C0
    b_n = texp * C1
    spec = Spec(
        body=(minn(maxx(Src0, Zero - b_n), b_n) + mb_n) - mb_n,
        reference=_ref,
    )
    row = _DO._CUSTOM_DVE_ROW_BASE + len(_DO.OPS)
    shas = {
        ver: DveOpSpec(
            name=name, opcode=row, uops=lower(spec, ver=ver), rd1_en=True
        ).sha(ver)
        for ver in ("v3", "v4")
    }
    op = _DO.DveOp(name, spec, subdim=False, uops_sha=shas)
    _DO.OPS.append(op)
    _DO.CUSTOM_DVE_SPECS[name] = spec
    _DO._SUB_OPCODE_FOR_NAME[name] = row
    return op


_BFP_OP = _register_bfp_op()


def _build_program():
    if "nc" in _CACHE:
        return _CACHE["nc"]
    nc = bacc.Bacc(
        "TRN2",
        target_bir_lowering=False,
        debug=False,
        enable_asserts=False,
        num_devices=N_CORES,
    )
    xu = nc.dram_tensor("xu", [NCHUNK, 128, FREE], F32, kind="ExternalInput")
    yo = nc.dram_tensor("yo", [NCHUNK, 128, FREE], FP8, kind="ExternalOutput")

    JH = J // 2  # 49: j-half for the ramp-in / drain chunks

    with tile.TileContext(nc) as tc:
        with (
            tc.tile_pool(name="xp", bufs=8) as xp,
            tc.tile_pool(name="wp", bufs=3) as wp,
            tc.tile_pool(name="op", bufs=3) as op_,
            tc.tile_pool(name="mp", bufs=6) as mp,
        ):
            xts, ws, mbs = {}, {}, {}

            def emit_load(k):
                if k >= NCHUNK or k in xts:
                    return
                xts[k] = xp.tile([128, J, CI], F32, name="xt", tag="xt")
                if k == 0:
                    # j-halves: the whole half-pipeline starts after half
                    # the bytes land (cuts the pipeline ramp-in)
                    for h in range(2):
                        nc.sync.dma_start(
                            xts[k][:, JH * h : JH * (h + 1), :],
                            bass.AP(
                                xu,
                                k * 128 * FREE + h * (FREE // 2),
                                [[FREE, 128], [1, FREE // 2]],
                            ),
                        )
                else:
                    nc.sync.dma_start(
                        xts[k][:],
                        bass.AP(xu, k * 128 * FREE, [[FREE, 128], [1, FREE]]),
                    )

            abs_done = set()

            def emit_abs(k, h=None):
                # |x| -> bf16; only the exponent of the maxabs survives.
                if k >= NCHUNK or (k, h) in abs_done or (k, None) in abs_done:
                    return
                abs_done.add((k, h))
                if k not in ws:
                    ws[k] = wp.tile([128, J, CI], BF16, name="w", tag="w")
                if h is None:
                    nc.scalar.activation(
                        ws[k][:], xts[k][:], mybir.ActivationFunctionType.Abs,
                    )
                else:
                    nc.scalar.activation(
                        ws[k][:, JH * h : JH * (h + 1), :],
                        xts[k][:, JH * h : JH * (h + 1), :],
                        mybir.ActivationFunctionType.Abs,
                    )

            def emit_reduce(k, h=None):
                # ONE op: m32[p, j] = max over the 32 contiguous ci
                if k >= NCHUNK:
                    return
                if k not in mbs:
                    mbs[k] = mp.tile([128, J], F32, name="m32", tag="m32")
                m32 = mbs[k]
                sl = slice(None) if h is None else slice(JH * h, JH * (h + 1))
                nc.vector.tensor_reduce(
                    out=m32[:, sl], in_=ws[k][:, sl, :],
                    axis=mybir.AxisListType.X, op=mybir.AluOpType.max,
                )

            def emit_quant(k, h=None):
                # the whole quantization in one custom DVE pass -> fp8
                if k >= NCHUNK:
                    return
                if k not in ots:
                    ots[k] = op_.tile([128, J, CI], FP8, name="ot", tag="ot")
                m32 = mbs[k]
                sl = slice(None) if h is None else slice(JH * h, JH * (h + 1))
                jn = J if h is None else JH
                nc.vector._custom_dve(
                    _BFP_OP,
                    out=ots[k][:, sl, :], in0=xts[k][:, sl, :],
                    in1=m32[:, sl].unsqueeze(2).broadcast_to([128, jn, CI]),
                    s0=3145728.0, s1=1.8,
                )
                off = 0 if h in (None, 0) else FREE // 2
                n = FREE if h is None else FREE // 2
                eng = nc.sync if k == NCHUNK - 1 else nc.gpsimd
                eng.dma_start(
                    bass.AP(yo, k * 128 * FREE + off, [[FREE, 128], [1, n]]),
                    ots[k][:, sl, :],
                )

            ots = {}

            # prologue: deep load lookahead; chunk 0 processed in j-halves
            for k in range(4):
                emit_load(k)
            emit_abs(0, h=0)
            emit_abs(0, h=1)
            emit_reduce(0, h=0)
            emit_quant(0, h=0)
            emit_reduce(0, h=1)
            emit_quant(0, h=1)
            emit_abs(1)

            for k in range(1, NCHUNK):
                emit_load(k + 3)
                emit_abs(k)
                if k == NCHUNK - 1:
                    # drain: j-halves so the last store starts after half
                    # the final custom pass
                    for h in range(2):
                        emit_reduce(k, h=h)
                        emit_quant(k, h=h)
                else:
                    emit_reduce(k)
                    emit_quant(k)
                emit_abs(k + 1)

    nc.compile()
    _CACHE["nc"] = nc
    return nc


def _permute_in(shard):
    # shard [4, 256, 3136] f32 -> [chunk][p 128][free],
    # p = img*32 + blk*4 + hwq, free = (j, ci), hw = hwq*784 + chunk*J + j
    t = shard.reshape(BPC, 8, CI, 4, NCHUNK, J)
    t = t.transpose(4, 0, 1, 3, 5, 2)  # [chunk, img, blk, hwq, j, ci]
    return np.ascontiguousarray(t).reshape(NCHUNK, 128, FREE)


def _permute_out(y):
    # y [chunk][p 128][free] f32 -> [4, 256, 3136]
    t = y.reshape(NCHUNK, BPC, 8, 4, J, CI)
    t = t.transpose(1, 2, 5, 3, 0, 4)  # [img, blk, ci, hwq, chunk, j]
    return np.ascontiguousarray(t).reshape(BPC, C, HW)


def kernel(activations=None, mantissa=3, blk=32, **_unused):
    x = np.ascontiguousarray(np.asarray(activations), dtype=np.float32)
    assert x.shape == (B, C, H, W), x.shape
    assert int(mantissa) == 3 and int(blk) == 32, (mantissa, blk)

    nc = _build_program()
    xr = x.reshape(B, C, HW)
    in_maps = [
        {"xu": _permute_in(xr[c * BPC : (c + 1) * BPC])} for c in range(N_CORES)
    ]
    res = run_bass_kernel_spmd(nc, in_maps, list(range(N_CORES))).results
    out = np.concatenate(
        [
            _permute_out(np.asarray(res[c]["yo"]).astype(np.float32)).reshape(
                BPC, C, H, W
            )
            for c in range(N_CORES)
        ],
        axis=0,
    )
    return out


def run_traced(activations):
    """test.py helper: run with NTFF tracing, return (out, BassKernelResults)."""
    x = np.ascontiguousarray(np.asarray(activations), dtype=np.float32)
    nc = _build_program()
    xr = x.reshape(B, C, HW)
    in_maps = [
        {"xu": _permute_in(xr[c * BPC : (c + 1) * BPC])} for c in range(N_CORES)
    ]
    r = run_bass_kernel_spmd(nc, in_maps, list(range(N_CORES)), trace=True)
    out = np.concatenate(
        [
            _permute_out(np.asarray(r.results[c]["yo"]).astype(np.float32)).reshape(
                BPC, C, H, W
            )
            for c in range(N_CORES)
        ],
        axis=0,
    )
    return out, r


# revision 16
# speedup vs baseline: 1.0111x; 1.0111x over previous
"""BFP (block floating point) activation quantization kernel for Trainium2.

Problem: x [32, 256, 56, 56] f32; per (batch, 32-channel block, h, w) the 32
channels share an exponent e = floor(log2(max |x|)); quantize each value to
q * 2^(e-2) with q = clip(round(x / 2^(e-2)), -7, 7)  (mantissa=3 bits).

Strategy (pure data-parallel over batch, 4 images per core on 8 cores):
  - The host permutes each core's shard into the SBUF layout
    [chunk 8][p 128][j 98][ci 32] (p = img*32 + blk*4 + hwq), so every DMA
    is a fully linear transfer AND the 32 channels of each block are the
    contiguous innermost run: the whole maxabs tree collapses into ONE
    vector.tensor_reduce(axis=X) per chunk (bf16 2x, input-bound).
  - Per chunk the engines split:
      ScalarE: w = |x| -> bf16 (the only full pass; exponent survives)
      Vector:  m32 = reduce_max(w) over ci (1 op), then ONE 8-stage custom
               DVE op computes the quantized output in a single f32 pass
               straight from m32:
                   t   = bits(m32) & POS_INF  (= 2^e; hard-wired +Inf mask)
                   Mb  = t * 3145728          (= MAGIC * 2^(e-2), exact)
                   B   = t * 1.8              (= 7.2*scale; any bound in
                                               (6.5, 7.5)*scale is exact)
                   out = (clip(x, -B, B) + Mb) - Mb    -> fp8 e4m3
               (the f32 +Mb add RNE-rounds to the scale grid whose step is
               the ulp at Mb's binade; -Mb is exact by Sterbenz)
      GpSimd:  only SWDGE store descriptor generation.
  - fp8 e4m3 output is exact (q has <= 3 mantissa bits, e in [-7,7] here)
    and halves store traffic; the host un-permutes/upconverts.
  - Chunk 0 and chunk 7 are processed in flat j-halves (the (j,ci) layout
    makes halves contiguous) to cut pipeline ramp-in and drain; the last
    chunk stores via the SP HWDGE ring right after each half's custom op.
"""

import numpy as np

import concourse.bass as bass
import concourse.tile as tile
from concourse import bacc, mybir
from concourse import dve_ops as _DO
from concourse.bass_utils import run_bass_kernel_spmd
from concourse.dve_spec import (
    C0, C1, Bin, Leaf, Spec, Src0, Src1, Zero, lower, maxx, minn,
)
from concourse.dve_uop import AluOp, DveOpSpec, InpSel

F32 = mybir.dt.float32
BF16 = mybir.dt.bfloat16
FP8 = mybir.dt.float8e4
I32 = mybir.dt.int32

N_CORES = 8
B, C, H, W = 32, 256, 56, 56
HW = H * W            # 3136
BPC = B // N_CORES    # 4 images per core
NCHUNK = 8
J = HW // 4 // NCHUNK  # 98 hw-inner elements per chunk
CI = 32               # channels per block
FREE = CI * J         # free elems per chunk per partition
MAGIC = 12582912.0    # 1.5 * 2**23: RNE round-to-integer magic for |v| < 2**22

_CACHE = {}


def _register_bfp_op():
    """Custom DVE op: the entire BFP quantization in one 8-stage pass.

        t   = in1 & 0x7F800000   (AND with the hard-wired +Inf source;
                                  in1 = maxabs > 0, so t = 2^e exactly)
        Mb  = t * s0             (s0 = 3145728 = MAGIC/4 -> MAGIC*2^(e-2);
                                  exact: power-of-two times 1.5*2^21)
        B   = t * s1             (s1 = 1.8 -> 7.2*2^(e-2); any bound in
                                  (6.5, 7.5)*scale gives exact clipping)
        out = (clip(in0, -B, B) + Mb) - Mb
    """
    name = "BFP_FUSED_QUANT"
    for op in _DO.OPS:
        if op.name == name:
            return op

    def _ref(in0, in1, s0, s1, imm2):
        # per-stage f32 rounding is load-bearing (the magic-add trick)
        f32 = np.float32
        tb = (in1.astype(f32).view(np.uint32) & 0x7F800000).view(f32)
        mb = (tb * f32(s0)).astype(f32)
        bb = (tb * f32(s1)).astype(f32)
        v = np.minimum(np.maximum(in0.astype(f32), -bb), bb).astype(f32)
        r = (v + mb).astype(f32)
        return (r - mb).astype(f32)

    texp = Bin(AluOp.BITWISE_AND, Src1, Leaf(InpSel.POS_INF))
    mb_n = texp * C0
    b_n = texp * C1
    spec = Spec(
        body=(minn(maxx(Src0, Zero - b_n), b_n) + mb_n) - mb_n,
        reference=_ref,
    )
    row = _DO._CUSTOM_DVE_ROW_BASE + len(_DO.OPS)
    shas = {
        ver: DveOpSpec(
            name=name, opcode=row, uops=lower(spec, ver=ver), rd1_en=True
        ).sha(ver)
        for ver in ("v3", "v4")
    }
    op = _DO.DveOp(name, spec, subdim=False, uops_sha=shas)
    _DO.OPS.append(op)
    _DO.CUSTOM_DVE_SPECS[name] = spec
    _DO._SUB_OPCODE_FOR_NAME[name] = row
    return op


_BFP_OP = _register_bfp_op()


def _build_program():
    if "nc" in _CACHE:
        return _CACHE["nc"]
    nc = bacc.Bacc(
        "TRN2",
        target_bir_lowering=False,
        debug=False,
        enable_asserts=False,
        num_devices=N_CORES,
    )
    xu = nc.dram_tensor("xu", [NCHUNK, 128, FREE], F32, kind="ExternalInput")
    yo = nc.dram_tensor("yo", [NCHUNK, 128, FREE], FP8, kind="ExternalOutput")

    JH = J // 2  # 49: j-half for the ramp-in / drain chunks

    with tile.TileContext(nc) as tc:
        with (
            tc.tile_pool(name="xp", bufs=8) as xp,
            tc.tile_pool(name="wp", bufs=3) as wp,
            tc.tile_pool(name="op", bufs=3) as op_,
            tc.tile_pool(name="mp", bufs=6) as mp,
        ):
            xts, ws, mbs = {}, {}, {}

            def emit_load(k):
                if k >= NCHUNK or k in xts:
                    return
                xts[k] = xp.tile([128, J, CI], F32, name="xt", tag="xt")
                if k == 0:
                    # j-halves: the whole half-pipeline starts after half
                    # the bytes land (cuts the pipeline ramp-in)
                    for h in range(2):
                        nc.sync.dma_start(
                            xts[k][:, JH * h : JH * (h + 1), :],
                            bass.AP(
                                xu,
                                k * 128 * FREE + h * (FREE // 2),
                                [[FREE, 128], [1, FREE // 2]],
                            ),
                        )
                else:
                    nc.sync.dma_start(
                        xts[k][:],
                        bass.AP(xu, k * 128 * FREE, [[FREE, 128], [1, FREE]]),
                    )

            abs_done = set()

            def emit_abs(k, h=None):
                # |x| -> bf16; only the exponent of the maxabs survives.
                if k >= NCHUNK or (k, h) in abs_done or (k, None) in abs_done:
                    return
                abs_done.add((k, h))
                if k not in ws:
                    ws[k] = wp.tile([128, J, CI], BF16, name="w", tag="w")
                if h is None:
                    nc.scalar.activation(
                        ws[k][:], xts[k][:], mybir.ActivationFunctionType.Abs,
                    )
                else:
                    nc.scalar.activation(
                        ws[k][:, JH * h : JH * (h + 1), :],
                        xts[k][:, JH * h : JH * (h + 1), :],
                        mybir.ActivationFunctionType.Abs,
                    )

            def emit_reduce(k, h=None):
                # ONE op: m32[p, j] = max over the 32 contiguous ci
                if k >= NCHUNK:
                    return
                if k not in mbs:
                    # bf16 reduce out: 16-bit I/O keeps the reduce in the
                    # packed perf mode; a tiny copy upconverts to f32 for
                    # the custom op (only the exponent bits matter, which
                    # the bf16->f32 upconvert preserves exactly).
                    mbs[k] = (
                        mp.tile([128, J], BF16, name="m16", tag="m16"),
                        mp.tile([128, J], F32, name="m32", tag="m32"),
                    )
                m16, m32 = mbs[k]
                sl = slice(None) if h is None else slice(JH * h, JH * (h + 1))
                nc.vector.tensor_reduce(
                    out=m16[:, sl], in_=ws[k][:, sl, :],
                    axis=mybir.AxisListType.X, op=mybir.AluOpType.max,
                )
                nc.vector.tensor_copy(m32[:, sl], m16[:, sl])

            def emit_quant(k, h=None):
                # the whole quantization in one custom DVE pass -> fp8
                if k >= NCHUNK:
                    return
                if k not in ots:
                    ots[k] = op_.tile([128, J, CI], FP8, name="ot", tag="ot")
                m32 = mbs[k][1]
                sl = slice(None) if h is None else slice(JH * h, JH * (h + 1))
                jn = J if h is None else JH
                nc.vector._custom_dve(
                    _BFP_OP,
                    out=ots[k][:, sl, :], in0=xts[k][:, sl, :],
                    in1=m32[:, sl].unsqueeze(2).broadcast_to([128, jn, CI]),
                    s0=3145728.0, s1=1.8,
                )
                off = 0 if h in (None, 0) else FREE // 2
                n = FREE if h is None else FREE // 2
                eng = nc.sync if k == NCHUNK - 1 else nc.gpsimd
                eng.dma_start(
                    bass.AP(yo, k * 128 * FREE + off, [[FREE, 128], [1, n]]),
                    ots[k][:, sl, :],
                )

            ots = {}

            # prologue: deep load lookahead; chunk 0 processed in j-halves
            for k in range(4):
                emit_load(k)
            emit_abs(0, h=0)
            emit_abs(0, h=1)
            emit_reduce(0, h=0)
            emit_quant(0, h=0)
            emit_reduce(0, h=1)
            emit_quant(0, h=1)
            emit_abs(1)

            for k in range(1, NCHUNK):
                emit_load(k + 3)
                emit_abs(k)
                if k == NCHUNK - 1:
                    # drain: j-halves so the last store starts after half
                    # the final custom pass
                    for h in range(2):
                        emit_reduce(k, h=h)
                        emit_quant(k, h=h)
                else:
                    emit_reduce(k)
                    emit_quant(k)
                emit_abs(k + 1)

    nc.compile()
    _CACHE["nc"] = nc
    return nc


def _permute_in(shard):
    # shard [4, 256, 3136] f32 -> [chunk][p 128][free],
    # p = img*32 + blk*4 + hwq, free = (j, ci), hw = hwq*784 + chunk*J + j
    t = shard.reshape(BPC, 8, CI, 4, NCHUNK, J)
    t = t.transpose(4, 0, 1, 3, 5, 2)  # [chunk, img, blk, hwq, j, ci]
    return np.ascontiguousarray(t).reshape(NCHUNK, 128, FREE)


def _permute_out(y):
    # y [chunk][p 128][free] f32 -> [4, 256, 3136]
    t = y.reshape(NCHUNK, BPC, 8, 4, J, CI)
    t = t.transpose(1, 2, 5, 3, 0, 4)  # [img, blk, ci, hwq, chunk, j]
    return np.ascontiguousarray(t).reshape(BPC, C, HW)


def kernel(activations=None, mantissa=3, blk=32, **_unused):
    x = np.ascontiguousarray(np.asarray(activations), dtype=np.float32)
    assert x.shape == (B, C, H, W), x.shape
    assert int(mantissa) == 3 and int(blk) == 32, (mantissa, blk)

    nc = _build_program()
    xr = x.reshape(B, C, HW)
    in_maps = [
        {"xu": _permute_in(xr[c * BPC : (c + 1) * BPC])} for c in range(N_CORES)
    ]
    res = run_bass_kernel_spmd(nc, in_maps, list(range(N_CORES))).results
    out = np.concatenate(
        [
            _permute_out(np.asarray(res[c]["yo"]).astype(np.float32)).reshape(
                BPC, C, H, W
            )
            for c in range(N_CORES)
        ],
        axis=0,
    )
    return out


def run_traced(activations):
    """test.py helper: run with NTFF tracing, return (out, BassKernelResults)."""
    x = np.ascontiguousarray(np.asarray(activations), dtype=np.float32)
    nc = _build_program()
    xr = x.reshape(B, C, HW)
    in_maps = [
        {"xu": _permute_in(xr[c * BPC : (c + 1) * BPC])} for c in range(N_CORES)
    ]
    r = run_bass_kernel_spmd(nc, in_maps, list(range(N_CORES)), trace=True)
    out = np.concatenate(
        [
            _permute_out(np.asarray(r.results[c]["yo"]).astype(np.float32)).reshape(
                BPC, C, H, W
            )
            for c in range(N_CORES)
        ],
        axis=0,
    )
    return out, r


# revision 17
# speedup vs baseline: 1.1293x; 1.1168x over previous
"""BFP (block floating point) activation quantization kernel for Trainium2.

Problem: x [32, 256, 56, 56] f32; per (batch, 32-channel block, h, w) the 32
channels share an exponent e = floor(log2(max |x|)); quantize each value to
q * 2^(e-2) with q = clip(round(x / 2^(e-2)), -7, 7)  (mantissa=3 bits).

Strategy (pure data-parallel over batch, 4 images per core on 8 cores):
  - The host permutes each core's shard into the SBUF layout
    [chunk 8][p 128][ci 32][j 98] (p = img*32 + blk*4 + hwq), so every DMA
    is a fully linear transfer and every tree level is a contiguous flat
    slice (bf16 2x perf mode).
  - Per chunk the engines split:
      ScalarE: w = |x| -> bf16 (the only full pass; exponent survives)
      Vector:  maxabs tree (bf16 max levels, flat APs; last level emits
               fp32 m32 [128, J]), then ONE 8-stage custom DVE op computes
               the quantized output in a single f32 pass straight from m32:
                   t   = bits(m32) & POS_INF  (= 2^e; hard-wired +Inf mask)
                   Mb  = t * 3145728          (= MAGIC * 2^(e-2), exact)
                   B   = t * 1.8              (= 7.2*scale; any bound in
                                               (6.5, 7.5)*scale is exact)
                   out = (clip(x, -B, B) + Mb) - Mb    -> fp8 e4m3
               (the f32 +Mb add RNE-rounds to the scale grid whose step is
               the ulp at Mb's binade; -Mb is exact by Sterbenz)
      GpSimd:  SWDGE store descriptor generation (and optionally the first
               tree levels, see GPSIMD_TREE).
  - fp8 e4m3 output is exact (q has <= 3 mantissa bits, e in [-7,7] here)
    and halves store traffic; the host un-permutes/upconverts.
  - Loads prefetch deep (8 buffers) on the SP HWDGE ring; chunk 0 loads
    and takes |x| in ci-halves to cut ramp-in; the last chunk runs the
    custom op and stores in ci-halves on the SP ring to cut the drain.
"""

import numpy as np

import concourse.bass as bass
import concourse.tile as tile
from concourse import bacc, mybir
from concourse import dve_ops as _DO
from concourse.bass_utils import run_bass_kernel_spmd
from concourse.dve_spec import (
    C0, C1, Bin, Leaf, Spec, Src0, Src1, Zero, lower, maxx, minn,
)
from concourse.dve_uop import AluOp, DveOpSpec, InpSel

F32 = mybir.dt.float32
BF16 = mybir.dt.bfloat16
FP8 = mybir.dt.float8e4
I32 = mybir.dt.int32

N_CORES = 8
B, C, H, W = 32, 256, 56, 56
HW = H * W            # 3136
BPC = B // N_CORES    # 4 images per core
NCHUNK = 8
J = HW // 4 // NCHUNK  # 98 hw-inner elements per chunk
CI = 32               # channels per block
FREE = CI * J         # free elems per chunk per partition
MAGIC = 12582912.0    # 1.5 * 2**23: RNE round-to-integer magic for |v| < 2**22
GPSIMD_TREE = 0       # how many leading tree levels run on GpSimd (0..2)

_CACHE = {}


def _register_bfp_op():
    """Custom DVE op: the entire BFP quantization in one 8-stage pass.

        t   = in1 & 0x7F800000   (AND with the hard-wired +Inf source;
                                  in1 = maxabs > 0, so t = 2^e exactly)
        Mb  = t * s0             (s0 = 3145728 = MAGIC/4 -> MAGIC*2^(e-2))
        B   = t * s1             (s1 = 1.8 -> 7.2*2^(e-2); any bound in
                                  (6.5, 7.5)*scale gives exact clipping)
        out = (clip(in0, -B, B) + Mb) - Mb
    """
    name = "BFP_FUSED_QUANT"
    for op in _DO.OPS:
        if op.name == name:
            return op

    def _ref(in0, in1, s0, s1, imm2):
        # per-stage f32 rounding is load-bearing (the magic-add trick)
        f32 = np.float32
        tb = (in1.astype(f32).view(np.uint32) & 0x7F800000).view(f32)
        mb = (tb * f32(s0)).astype(f32)
        bb = (tb * f32(s1)).astype(f32)
        v = np.minimum(np.maximum(in0.astype(f32), -bb), bb).astype(f32)
        r = (v + mb).astype(f32)
        return (r - mb).astype(f32)

    texp = Bin(AluOp.BITWISE_AND, Src1, Leaf(InpSel.POS_INF))
    mb_n = texp * C0
    b_n = texp * C1
    spec = Spec(
        body=(minn(maxx(Src0, Zero - b_n), b_n) + mb_n) - mb_n,
        reference=_ref,
    )
    row = _DO._CUSTOM_DVE_ROW_BASE + len(_DO.OPS)
    shas = {
        ver: DveOpSpec(
            name=name, opcode=row, uops=lower(spec, ver=ver), rd1_en=True
        ).sha(ver)
        for ver in ("v3", "v4")
    }
    op = _DO.DveOp(name, spec, subdim=False, uops_sha=shas)
    _DO.OPS.append(op)
    _DO.CUSTOM_DVE_SPECS[name] = spec
    _DO._SUB_OPCODE_FOR_NAME[name] = row
    return op


_BFP_OP = _register_bfp_op()


def _flat(ap):
    return ap.rearrange("p a b -> p (a b)")


def _build_program():
    if "nc" in _CACHE:
        return _CACHE["nc"]
    nc = bacc.Bacc(
        "TRN2",
        target_bir_lowering=False,
        debug=False,
        enable_asserts=False,
        num_devices=N_CORES,
    )
    xu = nc.dram_tensor("xu", [NCHUNK, 128, FREE], F32, kind="ExternalInput")
    yo = nc.dram_tensor("yo", [NCHUNK, 128, FREE], FP8, kind="ExternalOutput")

    with tile.TileContext(nc) as tc:
        with (
            tc.tile_pool(name="xp", bufs=8) as xp,
            tc.tile_pool(name="wp", bufs=3) as wp,
            tc.tile_pool(name="op", bufs=3) as op_,
            tc.tile_pool(name="mp", bufs=4) as mp,
        ):
            xts, ws, m32s, ots = {}, {}, {}, {}
            abs_done = set()

            def emit_load(k):
                if k >= NCHUNK or k in xts:
                    return
                xts[k] = xp.tile([128, CI, J], F32, name="xt", tag="xt")
                if k == 0:
                    for h in range(2):
                        nc.sync.dma_start(
                            xts[k][:, 16 * h : 16 * (h + 1), :],
                            bass.AP(
                                xu,
                                k * 128 * FREE + h * (FREE // 2),
                                [[FREE, 128], [1, FREE // 2]],
                            ),
                        )
                else:
                    nc.sync.dma_start(
                        xts[k][:],
                        bass.AP(xu, k * 128 * FREE, [[FREE, 128], [1, FREE]]),
                    )

            def emit_abs(k, h=None):
                # |x| -> bf16; only the exponent of the maxabs survives.
                if k >= NCHUNK or (k, h) in abs_done or (k, None) in abs_done:
                    return
                abs_done.add((k, h))
                if k not in ws:
                    ws[k] = wp.tile([128, CI, J], BF16, name="w", tag="w")
                if h is None:
                    nc.scalar.activation(
                        ws[k][:], xts[k][:], mybir.ActivationFunctionType.Abs,
                    )
                else:
                    nc.scalar.activation(
                        ws[k][:, 16 * h : 16 * (h + 1), :],
                        xts[k][:, 16 * h : 16 * (h + 1), :],
                        mybir.ActivationFunctionType.Abs,
                    )

            def emit_tree(k):
                # maxabs tree: bf16 max levels (flat APs, 2x mode); the
                # last level emits fp32 m32. Optionally the first levels
                # run on GpSimd to unload the Vector queue.
                if k >= NCHUNK:
                    return
                w = ws[k]
                for i, wdt in enumerate((16, 8, 4, 2)):
                    eng = nc.gpsimd if i < GPSIMD_TREE else nc.vector
                    eng.tensor_tensor(
                        out=_flat(w[:, 0:wdt, :]),
                        in0=_flat(w[:, 0:wdt, :]),
                        in1=_flat(w[:, wdt : 2 * wdt, :]),
                        op=mybir.AluOpType.max,
                    )
                m32s[k] = mp.tile([128, J], F32, name="m32", tag="m32")
                nc.vector.tensor_tensor(
                    out=m32s[k][:], in0=w[:, 0, :], in1=w[:, 1, :],
                    op=mybir.AluOpType.max,
                )

            def emit_quant(k, h=None):
                # the whole quantization in one custom DVE pass -> fp8
                if k >= NCHUNK:
                    return
                if k not in ots:
                    ots[k] = op_.tile([128, CI, J], FP8, name="ot", tag="ot")
                m32 = m32s[k]
                sl = slice(None) if h is None else slice(16 * h, 16 * (h + 1))
                cn = CI if h is None else 16
                nc.vector._custom_dve(
                    _BFP_OP,
                    out=ots[k][:, sl, :], in0=xts[k][:, sl, :],
                    in1=m32[:].unsqueeze(1).broadcast_to([128, cn, J]),
                    s0=3145728.0, s1=1.8,
                )
                off = 0 if h in (None, 0) else FREE // 2
                n = FREE if h is None else FREE // 2
                eng = nc.sync if k == NCHUNK - 1 else nc.gpsimd
                eng.dma_start(
                    bass.AP(yo, k * 128 * FREE + off, [[FREE, 128], [1, n]]),
                    ots[k][:, sl, :],
                )

            # prologue: deep prefetch; chunk 0 in ci-halves through |x|
            for k in range(4):
                emit_load(k)
            emit_abs(0, h=0)
            emit_abs(0, h=1)
            emit_abs(1)

            for k in range(NCHUNK):
                emit_load(k + 4)
                emit_abs(k + 1)
                emit_tree(k)
                # |x| of chunk k+2 queued on ScalarE before quant(k) runs
                # so ScalarE always stays a chunk ahead of the tree
                emit_abs(k + 2)
                if k == NCHUNK - 1:
                    # drain: ci-halves, each stored right after its half
                    emit_quant(k, h=0)
                    emit_quant(k, h=1)
                else:
                    emit_quant(k)

    nc.compile()
    _CACHE["nc"] = nc
    return nc


def _permute_in(shard):
    # shard [4, 256, 3136] f32 -> [chunk][p 128][free],
    # p = img*32 + blk*4 + hwq, free = (ci, j), hw = hwq*784 + chunk*J + j
    t = shard.reshape(BPC, 8, CI, 4, NCHUNK, J)
    t = t.transpose(4, 0, 1, 3, 2, 5)  # [chunk, img, blk, hwq, ci, j]
    return np.ascontiguousarray(t).reshape(NCHUNK, 128, FREE)


def _permute_out(y):
    # y [chunk][p 128][free] f32 -> [4, 256, 3136]
    t = y.reshape(NCHUNK, BPC, 8, 4, CI, J)
    t = t.transpose(1, 2, 4, 3, 0, 5)  # [img, blk, ci, hwq, chunk, j]
    return np.ascontiguousarray(t).reshape(BPC, C, HW)


def kernel(activations=None, mantissa=3, blk=32, **_unused):
    x = np.ascontiguousarray(np.asarray(activations), dtype=np.float32)
    assert x.shape == (B, C, H, W), x.shape
    assert int(mantissa) == 3 and int(blk) == 32, (mantissa, blk)

    nc = _build_program()
    xr = x.reshape(B, C, HW)
    in_maps = [
        {"xu": _permute_in(xr[c * BPC : (c + 1) * BPC])} for c in range(N_CORES)
    ]
    res = run_bass_kernel_spmd(nc, in_maps, list(range(N_CORES))).results
    out = np.concatenate(
        [
            _permute_out(np.asarray(res[c]["yo"]).astype(np.float32)).reshape(
                BPC, C, H, W
            )
            for c in range(N_CORES)
        ],
        axis=0,
    )
    return out


def run_traced(activations):
    """test.py helper: run with NTFF tracing, return (out, BassKernelResults)."""
    x = np.ascontiguousarray(np.asarray(activations), dtype=np.float32)
    nc = _build_program()
    xr = x.reshape(B, C, HW)
    in_maps = [
        {"xu": _permute_in(xr[c * BPC : (c + 1) * BPC])} for c in range(N_CORES)
    ]
    r = run_bass_kernel_spmd(nc, in_maps, list(range(N_CORES)), trace=True)
    out = np.concatenate(
        [
            _permute_out(np.asarray(r.results[c]["yo"]).astype(np.float32)).reshape(
                BPC, C, H, W
            )
            for c in range(N_CORES)
        ],
        axis=0,
    )
    return out, r
